# revision 14
# baseline (speedup 1.0000x reference)
"""Trainium2 Bass kernel for nn_BaselineModel_55018531061929 (2-layer HSTU-style
dense transformer, B=2 L=2048 D=1024 H=8, SiLU attention).

Sharding: sequence-parallel over tokens. 8 cores = 2 batches x 4 query chunks
of 512 tokens. Each core computes its tokens' K/V shards, AllGathers K/V
within its batch group of 4 cores, and runs full attention for its 512
queries over all 2048 keys (mask applied from input data). All matmuls run
as float32r (full-rate fp32) except the A@V matmul which uses bf16 weights.

Activations live feature-major in SBUF ([d partitions, tokens free]) so all
projections chain without transposes. RoPE uses a host-side permutation of
the Wq/Wk rows (de-interleaved even/odd) plus an on-device half-swap matmul.
"""

import os
import time

import numpy as np

B, L, D, H, NL = 2, 2048, 1024, 8, 2
HD = D // H
EPS = 1e-8
NC = 8
T = 512            # query tokens per core
DT = D // 128      # 8 d-tiles
KT = L // 128      # 16 key tiles
GROUPS = [[0, 1, 2, 3], [4, 5, 6, 7]]

_CACHE = {}


# --------------------------------------------------------------------------
# device program
# --------------------------------------------------------------------------

def _build_program():
    import concourse.bacc as bacc
    import concourse.mybir as mybir
    import concourse.tile as tile
    from concourse.masks import make_identity

    f32 = mybir.dt.float32
    f32r = mybir.dt.float32r
    bf16 = mybir.dt.bfloat16
    AF = mybir.ActivationFunctionType

    nc = bacc.Bacc("TRN2", target_bir_lowering=False, debug=False, num_devices=NC)

    # ---- I/O ----
    x_in = nc.dram_tensor("x_fm", [D, T], f32r, kind="ExternalInput")
    mask_in = nc.dram_tensor("mask_t", [L, T], bf16, kind="ExternalInput")
    cos_in = nc.dram_tensor("cosf", [128, T], f32, kind="ExternalInput")
    sin_in = nc.dram_tensor("sinf", [128, T], f32, kind="ExternalInput")
    psw_in = nc.dram_tensor("pswap", [128, 128], f32r, kind="ExternalInput")
    w_in = nc.dram_tensor("wstack", [7 * NL, 8, 128, DT, 128], f32r,
                          kind="ExternalInput")
    ones_in = nc.dram_tensor("onesf", [128, 128], f32r, kind="ExternalInput")
    b_in = nc.dram_tensor("bstack", [7 * NL, 128, 8], f32, kind="ExternalInput")
    lng_in = nc.dram_tensor("lng", [2 * NL + 1, 128, DT], f32, kind="ExternalInput")
    lnb_in = nc.dram_tensor("lnb", [2 * NL + 1, 128, DT], f32, kind="ExternalInput")
    out_t = nc.dram_tensor("out_fm", [D, T], f32r, kind="ExternalOutput")

    W_Q, W_K, W_U, W_V, W_O, W_1, W_2 = range(7)
    INV_SQRT_HD = float(1.0 / np.sqrt(HD))

    with tile.TileContext(nc) as tc:
        with (
            tc.tile_pool(name="const", bufs=1) as constp,
            tc.tile_pool(name="acts", bufs=1) as acts,
            tc.tile_pool(name="wcol", bufs=4) as wcolp,
            tc.tile_pool(name="tmp", bufs=4) as tmpp,
            tc.tile_pool(name="small", bufs=4) as smallp,
            tc.tile_pool(name="krp", bufs=2) as krp,
            tc.tile_pool(name="vrp", bufs=2) as vrp,
            tc.tile_pool(name="kfp", bufs=2) as kfp,
            tc.tile_pool(name="vfp", bufs=2) as vfp,
            tc.tile_pool(name="wtsp", bufs=3) as wtsp,
            tc.tile_pool(name="pmm", bufs=2, space="PSUM") as pmm,
            tc.tile_pool(name="psc", bufs=2, space="PSUM") as pscp,
            tc.tile_pool(name="pav", bufs=2, space="PSUM") as pavp,
            tc.tile_pool(name="dram", bufs=1, space="DRAM") as dramp,
        ):
            # ---- constants ----
            x_sb = constp.tile([128, DT, T], f32r, name="x_sb")
            nc.sync.dma_start(x_sb[:], x_in.ap().rearrange("(dt p) t -> p dt t", p=128))
            mask_sb = constp.tile([128, KT, T], bf16, name="mask_sb")
            nc.sync.dma_start(mask_sb[:],
                              mask_in.ap().rearrange("(kt p) t -> p kt t", p=128))
            cos_sb = constp.tile([128, T], f32, name="cos_sb")
            nc.sync.dma_start(cos_sb[:], cos_in[:])
            sin_sb = constp.tile([128, T], f32, name="sin_sb")
            nc.sync.dma_start(sin_sb[:], sin_in[:])
            psw_sb = constp.tile([128, 128], f32r, name="psw_sb")
            nc.sync.dma_start(psw_sb[:], psw_in[:])
            bcol_sb = constp.tile([128, 7 * NL, 8], f32, name="bcol_sb")
            nc.sync.dma_start(bcol_sb[:], b_in.ap().rearrange("w p c -> p w c"))
            lng_sb = constp.tile([128, 2 * NL + 1, DT], f32, name="lng_sb")
            nc.sync.dma_start(lng_sb[:], lng_in.ap().rearrange("w p c -> p w c"))
            lnb_sb = constp.tile([128, 2 * NL + 1, DT], f32, name="lnb_sb")
            nc.sync.dma_start(lnb_sb[:], lnb_in.ap().rearrange("w p c -> p w c"))
            ident = constp.tile([128, 128], f32, name="ident")
            make_identity(nc, ident)
            ones_sb = constp.tile([128, 128], f32r, name="ones_sb")
            nc.sync.dma_start(ones_sb[:], ones_in[:])
            ones_col = ones_sb[:, 0:1]
            ones_row = ones_sb[0:1, :]
            eps_col = constp.tile([128, 1], f32, name="eps_col")
            nc.vector.memset(eps_col[:], EPS)

            # ---- collective buffers ----
            agin_k = [dramp.tile([8, 128, T], f32r, name=f"agin_k{l}")
                      for l in range(NL)]
            agout_k = [dramp.tile([4, 8, 128, T], f32r, name=f"agout_k{l}")
                       for l in range(NL)]
            agin_v = [dramp.tile([8, T, 128], bf16, name=f"agin_v{l}")
                      for l in range(NL)]
            agout_v = [dramp.tile([4, 8, T, 128], bf16, name=f"agout_v{l}")
                       for l in range(NL)]

            def load_wcol(widx, ot):
                w = wcolp.tile([128, DT, 128], f32r, name="wct", tag="wct")
                nc.sync.dma_start(w[:], w_in[widx, ot])
                return w

            def layernorm(idx):
                """Normalize x_sb -> new 'bigA' tile, using ln row idx."""
                ps_sum = pmm.tile([1, T], f32, name="ps_sum", tag="pmm")
                ps_sq = pmm.tile([1, T], f32, name="ps_sq", tag="pmm")
                for dt in range(DT):
                    sqv = tmpp.tile([128, T], f32r, name="sqv", tag="tmp")
                    nc.scalar.square(sqv[:], x_sb[:, dt, :])
                    nc.tensor.matmul(ps_sum[:], ones_col[:], x_sb[:, dt, :],
                                     start=dt == 0, stop=dt == DT - 1)
                    nc.tensor.matmul(ps_sq[:], ones_col[:], sqv[:],
                                     start=dt == 0, stop=dt == DT - 1)
                s_mean = smallp.tile([1, T], f32, name="s_mean", tag="sm")
                nc.vector.tensor_scalar_mul(s_mean[:], ps_sum[:], 1.0 / D)
                s_var = smallp.tile([1, T], f32, name="s_var", tag="sm")
                nc.vector.tensor_scalar_mul(s_var[:], ps_sq[:], 1.0 / D)
                s_msq = smallp.tile([1, T], f32, name="s_msq", tag="sm")
                nc.vector.tensor_mul(s_msq[:], s_mean[:], s_mean[:])
                nc.vector.tensor_sub(s_var[:], s_var[:], s_msq[:])
                s_std = smallp.tile([1, T], f32, name="s_std", tag="sm")
                nc.scalar.activation(s_std[:], s_var[:], AF.Sqrt, bias=eps_col[:1])
                s_istd = smallp.tile([1, T], f32r, name="s_istd", tag="sm")
                with nc.allow_low_precision(reason="f32r is full-width fp32"):
                    nc.vector.reciprocal(s_istd[:], s_std[:])
                s_ms = smallp.tile([1, T], f32r, name="s_ms", tag="sm")
                nc.vector.tensor_mul(s_ms[:], s_mean[:], s_istd[:])
                b_istd = pmm.tile([128, T], f32, name="b_istd", tag="pmm")
                nc.tensor.matmul(b_istd[:], ones_row[:], s_istd[:],
                                 start=True, stop=True)
                b_ms = pmm.tile([128, T], f32, name="b_ms", tag="pmm")
                nc.tensor.matmul(b_ms[:], ones_row[:], s_ms[:],
                                 start=True, stop=True)
                h = acts.tile([128, DT, T], f32r, name="h", tag="bigA")
                for dt in range(DT):
                    t1 = tmpp.tile([128, T], f32, name="t1", tag="tmp")
                    nc.vector.tensor_mul(t1[:], x_sb[:, dt, :], b_istd[:])
                    nc.vector.tensor_sub(t1[:], t1[:], b_ms[:])
                    nc.scalar.activation(h[:, dt, :], t1[:], AF.Identity,
                                         bias=lnb_sb[:, idx, dt:dt + 1],
                                         scale=lng_sb[:, idx, dt:dt + 1])
                return h

            def proj_psum(widx, ot, rhs_tile):
                """One [128, T] psum = sum_dt W[widx,ot,:,dt].T @ rhs[:,dt,:]."""
                w = load_wcol(widx, ot)
                ps = pmm.tile([128, T], f32, name="ps_p", tag="pmm")
                for dt in range(DT):
                    nc.tensor.matmul(ps[:], w[:, dt, :], rhs_tile[:, dt, :],
                                     start=dt == 0, stop=dt == DT - 1)
                return ps

            def rope_into(dst_ap, src_tile):
                """dst = src*cosf + (pswap@src)*sinf."""
                psw = pmm.tile([128, T], f32, name="psw_ps", tag="pmm")
                nc.tensor.matmul(psw[:], psw_sb[:], src_tile[:],
                                 start=True, stop=True)
                nc.vector.tensor_mul(dst_ap, src_tile[:], cos_sb[:])
                t2 = tmpp.tile([128, T], f32, name="rt2", tag="tmp")
                nc.vector.tensor_mul(t2[:], psw[:], sin_sb[:])
                nc.vector.tensor_add(dst_ap, dst_ap, t2[:])

            for layer in range(NL):
                wofs = 7 * layer
                h = layernorm(2 * layer)

                # ---- K projection + rope -> AG input ----
                for ot in range(H):
                    ps = proj_psum(wofs + W_K, ot, h)
                    ktmp = tmpp.tile([128, T], f32r, name="ktmp", tag="tmp")
                    nc.scalar.activation(ktmp[:], ps[:], AF.Identity,
                                         bias=bcol_sb[:, wofs + W_K, ot:ot + 1])
                    kr = krp.tile([128, T], f32r, name="kr", tag="kr")
                    rope_into(kr[:], ktmp)
                    nc.sync.dma_start(agin_k[layer][ot], kr[:])
                nc.gpsimd.collective_compute(
                    "AllGather", mybir.AluOpType.bypass,
                    replica_groups=GROUPS,
                    ins=[agin_k[layer].opt()], outs=[agout_k[layer].opt()],
                )

                # ---- V projection + producer-side transpose -> AG input ----
                for ot in range(H):
                    ps = proj_psum(wofs + W_V, ot, h)
                    vtmp = tmpp.tile([128, T], f32, name="vtmp", tag="tmp")
                    nc.scalar.activation(vtmp[:], ps[:], AF.Identity,
                                         bias=bcol_sb[:, wofs + W_V, ot:ot + 1])
                    vr = vrp.tile([128, 4, 128], bf16, name="vr", tag="vr")
                    for j in range(4):
                        pst = pmm.tile([128, 128], f32, name="pst", tag="pmm")
                        nc.tensor.transpose(pst[:], vtmp[:, j * 128:(j + 1) * 128],
                                            ident[:])
                        nc.vector.tensor_copy(vr[:, j, :], pst[:])
                    nc.sync.dma_start(
                        agin_v[layer][ot].rearrange("(j p) hd -> p j hd", p=128),
                        vr[:])
                nc.gpsimd.collective_compute(
                    "AllGather", mybir.AluOpType.bypass,
                    replica_groups=GROUPS,
                    ins=[agin_v[layer].opt()], outs=[agout_v[layer].opt()],
                )

                # ---- Q (rope) and U projections ----
                q_sb = acts.tile([128, H, T], f32r, name="q_sb", tag="q")
                for ot in range(H):
                    ps = proj_psum(wofs + W_Q, ot, h)
                    qtmp = tmpp.tile([128, T], f32r, name="qtmp", tag="tmp")
                    nc.scalar.activation(qtmp[:], ps[:], AF.Identity,
                                         bias=bcol_sb[:, wofs + W_Q, ot:ot + 1])
                    rope_into(q_sb[:, ot, :], qtmp)
                u_sb = acts.tile([128, H, T], f32, name="u_sb", tag="u")
                for ot in range(H):
                    ps = proj_psum(wofs + W_U, ot, h)
                    nc.scalar.activation(u_sb[:, ot, :], ps[:], AF.Identity,
                                         bias=bcol_sb[:, wofs + W_U, ot:ot + 1])

                # ---- attention ----
                a_sb = acts.tile([128, H, T], f32r, name="a_sb", tag="bigA")
                for hh in range(H):
                    kf = kfp.tile([128, L], f32r, name="kf", tag="kf")
                    nc.sync.dma_start(
                        kf[:].rearrange("p (r t) -> p r t", r=4),
                        agout_k[layer][:, hh].rearrange("r p t -> p r t"))
                    vf = vfp.tile([128, KT, 128], bf16, name="vf", tag="vf")
                    for rr in range(4):
                        nc.sync.dma_start(
                            vf[:, 4 * rr:4 * rr + 4, :],
                            agout_v[layer][rr, hh].rearrange(
                                "(s p) hd -> p s hd", p=128))
                    pav = pavp.tile([128, T], f32, name="pav", tag="pav")
                    for ktp in range(KT // 2):
                        psc = pscp.tile([128, 2, T], f32, name="psc", tag="psc")
                        for j in range(2):
                            kt = 2 * ktp + j
                            nc.tensor.matmul(
                                psc[:, j, :],
                                kf[:, kt * 128:(kt + 1) * 128],
                                q_sb[:, hh, :], start=True, stop=True)
                        wt = wtsp.tile([128, 2, T], bf16, name="wt", tag="wt")
                        nc.scalar.activation(wt[:], psc[:], AF.Silu,
                                             scale=INV_SQRT_HD)
                        nc.vector.tensor_mul(
                            wt[:], wt[:], mask_sb[:, 2 * ktp:2 * ktp + 2, :])
                        for j in range(2):
                            kt = 2 * ktp + j
                            nc.tensor.matmul(pav[:], vf[:, kt, :], wt[:, j, :],
                                             start=kt == 0, stop=kt == KT - 1)
                    nc.vector.tensor_mul(a_sb[:, hh, :], pav[:], u_sb[:, hh, :])

                # ---- output projection + residual ----
                for ot in range(DT):
                    ps = proj_psum(wofs + W_O, ot, a_sb)
                    otmp = tmpp.tile([128, T], f32, name="otmp", tag="tmp")
                    nc.vector.tensor_scalar_add(
                        otmp[:], ps[:], bcol_sb[:, wofs + W_O, ot:ot + 1])
                    nc.vector.tensor_add(x_sb[:, ot, :], x_sb[:, ot, :], otmp[:])

                # ---- FFN ----
                h2 = layernorm(2 * layer + 1)
                p_sb = acts.tile([128, DT, T], f32, name="p_sb", tag="p")
                for ot in range(DT):
                    ps = proj_psum(wofs + W_1, ot, h2)
                    nc.scalar.activation(p_sb[:, ot, :], ps[:], AF.Identity,
                                         bias=bcol_sb[:, wofs + W_1, ot:ot + 1])
                gp = acts.tile([128, DT, T], f32r, name="gp", tag="bigA")
                for ot in range(DT):
                    sp = tmpp.tile([128, T], f32, name="sp", tag="tmp")
                    nc.scalar.activation(sp[:], p_sb[:, ot, :], AF.Silu)
                    nc.vector.tensor_mul(gp[:, ot, :], p_sb[:, ot, :], sp[:])
                for ot in range(DT):
                    ps = proj_psum(wofs + W_2, ot, gp)
                    ftmp = tmpp.tile([128, T], f32, name="ftmp", tag="tmp")
                    nc.vector.tensor_scalar_add(
                        ftmp[:], ps[:], bcol_sb[:, wofs + W_2, ot:ot + 1])
                    nc.vector.tensor_add(x_sb[:, ot, :], x_sb[:, ot, :], ftmp[:])

            # ---- final layernorm + output ----
            hf = layernorm(2 * NL)
            nc.sync.dma_start(
                out_t.ap().rearrange("(dt p) t -> p dt t", p=128), hf[:])

    nc.compile()
    return nc


# --------------------------------------------------------------------------
# host-side preparation
# --------------------------------------------------------------------------

def _host_prep(inputs):
    import ml_dtypes
    bf16 = ml_dtypes.bfloat16

    seqs = np.asarray(inputs["seqs"], np.float32)
    mask = np.asarray(inputs["attn_mask"])

    perm128 = np.concatenate([np.arange(0, 128, 2), np.arange(1, 128, 2)])
    perm_full = np.concatenate([h * 128 + perm128 for h in range(H)])

    def wprep(W):
        WT = np.ascontiguousarray(W.T)
        return np.ascontiguousarray(
            WT.reshape(DT, 128, 8, 128).transpose(2, 1, 0, 3))

    def bcolv(b):
        return np.ascontiguousarray(b.reshape(8, 128).T)

    def lncol(v):
        return np.ascontiguousarray(v.reshape(DT, 128).T)

    wstack, bstack = [], []
    for i in range(NL):
        for nm in ["Wq", "Wk", "Wu", "Wv", "Wo", "W1", "W2"]:
            Wm = np.asarray(inputs[nm][i], np.float32)
            bm = np.asarray(inputs["b" + nm[1:].lower()][i], np.float32)
            if nm in ("Wq", "Wk"):
                Wm = Wm[perm_full]
                bm = bm[perm_full]
            wstack.append(wprep(Wm))
            bstack.append(bcolv(bm))
    wstack = np.ascontiguousarray(np.stack(wstack), dtype=np.float32)
    bstack = np.ascontiguousarray(np.stack(bstack), dtype=np.float32)

    lng = np.stack([lncol(np.asarray(inputs["ln1_g"][0], np.float32)),
                    lncol(np.asarray(inputs["ln2_g"][0], np.float32)),
                    lncol(np.asarray(inputs["ln1_g"][1], np.float32)),
                    lncol(np.asarray(inputs["ln2_g"][1], np.float32)),
                    lncol(np.asarray(inputs["lnf_g"], np.float32))])
    lnb = np.stack([lncol(np.asarray(inputs["ln1_b"][0], np.float32)),
                    lncol(np.asarray(inputs["ln2_b"][0], np.float32)),
                    lncol(np.asarray(inputs["ln1_b"][1], np.float32)),
                    lncol(np.asarray(inputs["ln2_b"][1], np.float32)),
                    lncol(np.asarray(inputs["lnf_b"], np.float32))])
    lng = np.ascontiguousarray(lng, dtype=np.float32)
    lnb = np.ascontiguousarray(lnb, dtype=np.float32)

    pos = np.arange(L, dtype=np.float32)
    ar = np.arange(0, HD, 2).astype(np.float32) / np.float32(HD)
    freqs = np.float32(1.0) / np.power(np.float32(10000.0), ar, dtype=np.float32)
    ang = pos[:, None] * freqs[None, :]
    sin_full, cos_full = np.sin(ang).astype(np.float32), np.cos(ang).astype(np.float32)

    pswap = np.zeros((128, 128), np.float32)
    for i in range(64):
        pswap[i, 64 + i] = 1.0
        pswap[64 + i, i] = 1.0

    in_maps = []
    for c in range(NC):
        b_idx, q0 = c // 4, (c % 4) * T
        cos_t = cos_full[q0:q0 + T].T
        sin_t = sin_full[q0:q0 + T].T
        in_maps.append({
            "x_fm": np.ascontiguousarray(seqs[b_idx, q0:q0 + T].T),
            "mask_t": np.ascontiguousarray(
                mask[b_idx, q0:q0 + T].T.astype(np.float32)).astype(bf16),
            "cosf": np.ascontiguousarray(np.concatenate([cos_t, cos_t], 0)),
            "sinf": np.ascontiguousarray(np.concatenate([-sin_t, sin_t], 0)),
            "pswap": pswap, "onesf": np.ones((128, 128), np.float32),
            "wstack": wstack, "bstack": bstack, "lng": lng, "lnb": lnb,
        })
    return in_maps


def _get_program():
    if "nc" not in _CACHE:
        os.environ.setdefault("JAX_PLATFORMS", "")
        _CACHE["nc"] = _build_program()
    return _CACHE["nc"]


def _run(in_maps):
    from concourse.bass_utils import run_bass_kernel_spmd
    nc = _get_program()
    res = run_bass_kernel_spmd(nc, in_maps, core_ids=list(range(NC)))
    return res.results


def kernel(**inputs):
    in_maps = _host_prep(inputs)
    results = _run(in_maps)
    out = np.zeros((B, L, D), np.float32)
    for c in range(NC):
        b_idx, q0 = c // 4, (c % 4) * T
        out[b_idx, q0:q0 + T] = results[c]["out_fm"].T
    return out


# --------------------------------------------------------------------------
# benchmarking helper (used by test.py, not by the grading harness)
# --------------------------------------------------------------------------

def bench(inputs, iters=10):
    """Returns (outputs, per-call wall times) using a cached jitted callable."""
    import jax
    from jax.experimental.shard_map import shard_map
    from jax.sharding import Mesh, PartitionSpec
    import concourse.bass2jax as bass2jax
    import concourse.mybir as mybir

    nc = _get_program()
    in_maps = _host_prep(inputs)
    bass2jax.install_neuronx_cc_hook()

    partition_name = (nc.partition_id_tensor.name
                      if nc.partition_id_tensor else None)
    in_names, out_names, out_avals, zero_outs = [], [], [], []
    for alloc in nc.m.functions[0].allocations:
        if not isinstance(alloc, mybir.MemoryLocationSet):
            continue
        name = alloc.memorylocations[0].name
        if alloc.kind == "ExternalInput":
            if name != partition_name:
                in_names.append(name)
        elif alloc.kind == "ExternalOutput":
            out_names.append(name)
            shape = tuple(alloc.tensor_shape)
            dtype = mybir.dt.np(alloc.dtype)
            out_avals.append(jax.core.ShapedArray(shape, dtype))
            zero_outs.append(np.zeros(shape, dtype))
    n_params = len(in_names)
    all_names = in_names + out_names
    if partition_name is not None:
        all_names = all_names + [partition_name]

    def _body(*args):
        operands = list(args)
        if partition_name is not None:
            operands.append(bass2jax.partition_id_tensor())
        outs = bass2jax._bass_exec_p.bind(
            *operands,
            out_avals=tuple(out_avals),
            in_names=tuple(all_names),
            out_names=tuple(out_names),
            lowering_input_output_aliases=(),
            sim_require_finite=True,
            sim_require_nnan=True,
            nc=nc,
        )
        return tuple(outs)

    devices = jax.devices()[:NC]
    mesh = Mesh(np.asarray(devices), ("core",))
    n_outs = len(out_names)
    sharded = jax.jit(
        shard_map(_body, mesh=mesh,
                  in_specs=(PartitionSpec("core"),) * (n_params + n_outs),
                  out_specs=(PartitionSpec("core"),) * n_outs,
                  check_rep=False),
        donate_argnums=tuple(range(n_params, n_params + n_outs)),
        keep_unused=True,
    )
    concat_in = [np.concatenate([np.asarray(in_maps[c][nm])[None] for c in range(NC)],
                                axis=0).reshape(NC * in_maps[0][nm].shape[0],
                                                *in_maps[0][nm].shape[1:])
                 for nm in in_names]
    from jax.sharding import NamedSharding
    shard = NamedSharding(mesh, PartitionSpec("core"))
    in_arrs = [jax.device_put(a, shard) for a in concat_in]

    def fresh_zeros():
        return [jax.device_put(np.zeros((NC * z.shape[0], *z.shape[1:]), z.dtype),
                               shard) for z in zero_outs]

    # warmup (traces + compiles + first exec)
    outs = sharded(*in_arrs, *fresh_zeros())
    jax.block_until_ready(outs)

    times = []
    final = outs
    for _ in range(iters):
        zs = fresh_zeros()
        jax.block_until_ready(zs)
        t0 = time.perf_counter()
        final = sharded(*in_arrs, *zs)
        jax.block_until_ready(final)
        times.append(time.perf_counter() - t0)

    out = np.zeros((B, L, D), np.float32)
    arr0 = np.asarray(final[out_names.index("out_fm")]).reshape(NC, D, T)
    for c in range(NC):
        b_idx, q0 = c // 4, (c % 4) * T
        out[b_idx, q0:q0 + T] = arr0[c].T
    return out, times
